# revision 67
# baseline (speedup 1.0000x reference)
"""B-spline (clamped) surface evaluation on 8 Trainium2 cores.

Math: out[u, v, :] = sum_{a,b} Bu[u,a] * Bv[v,b] * P[su[u]-p+a, sv[v]-p+b, :]

Host precomputes the tiny Cox-de-Boor basis, scatters it into dense matrices
Au [Nu, 64], Av [Nv, 64], and folds the small control-point contraction
T[u, j, d] = sum_i Au[u, i] P[i, j, d] (25M MACs, fp64 on host). The device
then does the dominant contraction (768M MACs):

  S[u, v, d] = sum_j T[u, j, d] * Av[v, j]       (TensorEngine matmuls)

The harness tolerance is 2e-2 rel; a single bf16 pass over Av (the rhs)
lands ~2.4e-3, so no second (Av_lo) pass is needed. The weights keep the
hi/lo split packed on the K axis (lhsT = [T_hi; T_lo], rhs = [Av; Av]) —
that costs nothing (matmul time scales with N only) and keeps the full
128x128 PE array active, which the HAM clock-gate needs to un-throttle the
PE from 1.2 to 2.4 GHz (observed: K=64 matmuls stay cold forever).

Schedule notes (from NTFF analysis on this platform):
- THE load-bearing discovery: a single outstanding SWDGE DMA streams at
  ~350-550 GB/s across all 16 SDMA engines, but the moment a second DMA
  coexists anywhere in the subsystem, throughput latches to ~83 GB/s
  (three engines' worth) until the rings drain. HWDGE stores never engage
  more than 3 engines at all. The output is therefore written by exactly
  TWO SWDGE DMAs (1 MB head once planes 0-1 are ready, then the 2 MB
  rest), the second chained on the first's COMPLETION semaphore, and the
  head's ~2 us completion-receipt hides under the remaining copies.
- Every DMA completion waits on all 16 per-engine semaphore increments;
  the last one lands ~2 us after the data. Inputs go out as a small head
  chunk (tt0: first weight group; av0: first v-tile) that unblocks the
  first matmuls earlier, with the rest arriving on the other HWDGE ring
  in parallel (merged/single-DMA input variants measured slower).
- Warm-up matmuls on a zeroed scratch tile bridge the preamble->input gap
  so the PE HAM activity window is busy before the real stream starts.
- PSUM->SBUF cast-copies alternate DVE/ACT (~690 ns per [128,512] fp32
  tile each; this paces the middle phase at ~8.4 us). GPSIMD cannot read
  PSUM on TRN2.
- Output DRAM layout is a straight SBUF image, partition-major: DRAM row
  q holds partition q's six plane-blocks [p0|..|p5] (plane p = 3*u_tile
  + d, row u = 128*u_tile + q), so every partition is one contiguous
  24 KB chunk.

Sharding: data-parallel over u. Each core computes a [251, 2001, 3] slab.
"""

import numpy as np

N_CTRL = 64
N_EVAL = 2001
N_CORES = 8
NU_SHARD = 251  # ceil(2001 / 8); 8 * 251 = 2008 (last 7 rows are zero padding)
U_TILES = [(0, 128), (128, NU_SHARD - 128)]
V_TILE = 512
N_WARMUP_MM = 30  # short N=128 streams; chain sized to end ~when input lands

_CACHE = {}


def _clamped_knots(p, n_ctrl, dtype=np.float64):
    n_internal = n_ctrl - p - 1
    internal = np.linspace(0.0, 1.0, n_internal + 2, dtype=dtype)[1:-1]
    return np.concatenate(
        [np.zeros(p + 1, dtype), internal, np.ones(p + 1, dtype)]
    )


def _dense_basis(params, p, n_ctrl):
    """Dense basis matrix A [len(params), n_ctrl], float64, with
    A[k, span-p+a] = B[k, a] (Cox-de-Boor, NURBS book A2.2)."""
    knots = _clamped_knots(p, n_ctrl)
    u = np.asarray(params, np.float64)
    spans = np.clip(np.searchsorted(knots, u, side="right") - 1, p, n_ctrl - 1)
    Ns = [np.ones_like(u)]
    left = {}
    right = {}
    for j in range(1, p + 1):
        left[j] = u - knots[spans + 1 - j]
        right[j] = knots[spans + j] - u
        saved = np.zeros_like(u)
        new = []
        for r in range(j):
            temp = Ns[r] / (right[r + 1] + left[j - r])
            new.append(saved + right[r + 1] * temp)
            saved = left[j - r] * temp
        new.append(saved)
        Ns = new
    B = np.stack(Ns, axis=-1)  # [N, p+1]
    A = np.zeros((len(u), n_ctrl), np.float64)
    rows = np.arange(len(u))[:, None]
    cols = spans[:, None] - p + np.arange(p + 1)[None, :]
    A[rows, cols] = B
    return A


def _split_bf16(a):
    """fp32 array -> (hi, lo) bf16 arrays with hi + lo ~= a (~2^-17 rel)."""
    import ml_dtypes

    a = np.ascontiguousarray(a, np.float32)
    hi = a.astype(ml_dtypes.bfloat16)
    lo = (a - hi.astype(np.float32)).astype(ml_dtypes.bfloat16)
    return hi, lo


def _stack_hilo(a):
    """[64, N] fp32 -> [128, N] bf16 with rows 0-63 = hi, 64-127 = lo."""
    hi, lo = _split_bf16(a)
    return np.ascontiguousarray(np.concatenate([hi, lo], axis=0))


def _dup_halves_bf16(a):
    """[64, N] fp32 -> [128, N] bf16 with the same data in both halves."""
    import ml_dtypes

    h = np.asarray(a, np.float32).astype(ml_dtypes.bfloat16)
    return np.ascontiguousarray(np.concatenate([h, h], axis=0))


def _build_device():
    if "nc" in _CACHE:
        return _CACHE["nc"]

    import concourse.mybir as mybir
    import concourse.tile as tile
    from concourse import bacc

    f32 = mybir.dt.float32
    bf16 = mybir.dt.bfloat16
    nc = bacc.Bacc(
        "TRN2", target_bir_lowering=False, debug=False, num_devices=N_CORES,
    )
    # ONE merged input DMA ([tt | av], 704 KB) on SWDGE ring 0: a solo
    # SWDGE DMA streams at full rate (~400 GB/s; HWDGE loads crawl at
    # ~150 GB/s and parallel split pieces trip the two-outstanding latch
    # with core-dependent stragglers). Data lands ~8.9 us, completion sem
    # ~11.2 us, UNIFORM across cores — and the graded time is the max.
    in_specs = [
        ("inp", 3 * NU_SHARD + N_EVAL),
    ]
    ins = {
        name: nc.dram_tensor(name, [128, cols], bf16, kind="ExternalInput").ap()
        for name, cols in in_specs
    }
    # ONE 3 MB output DMA at the end, partition-major: DRAM row q holds
    # partition q's six plane-blocks [p0|..|p5] (plane p = 3*u_tile + d,
    # row u = 128*u_tile + q), i.e. a straight SBUF image. A single
    # outstanding SWDGE DMA sustains ~350-430 GB/s, but the moment two
    # coexist in the ring the SDMA subsystem latches to ~83 GB/s (three
    # engines' worth) until it drains — one big DMA beats any pipelining.
    out_h = nc.dram_tensor(
        "out", [128, 6 * N_EVAL], bf16, kind="ExternalOutput"
    ).ap()

    with tile.TileContext(nc) as tc:
        with (
            tc.tile_pool(name="consts", bufs=1) as consts,
            tc.tile_pool(name="ps2", bufs=7, space="PSUM") as ps2,
            tc.tile_pool(name="warm", bufs=1, space="PSUM") as ps_warm,
            tc.tile_pool(name="obuf", bufs=2) as obuf,
        ):
            # PE warm-up on a zeroed scratch tile while the input DMA flies
            # (Tile refuses read-never-written tiles, so the memset stays).
            # The tile is deliberately tiny ([128,128]): its memset takes
            # ~0.1 us instead of ~0.5, so the warm-up chain starts ~1.1 us
            # earlier and the HAM un-throttle arrives sooner; short N=128
            # streams keep the PE ~fully busy until the input semaphore.
            warm_in = consts.tile([128, 128], bf16, tag="warm", name="warm")
            nc.vector.memset(warm_in, 0)
            pw = ps_warm.tile([128, V_TILE], f32, tag="psw")
            warm_prev = None
            for i in range(N_WARMUP_MM):
                wmm = nc.tensor.matmul(
                    pw[:128, :128], warm_in, warm_in, start=True, stop=True
                )
                if warm_prev is not None:
                    tile.add_dep_helper(
                        wmm.ins, warm_prev.ins, sync=False, reason="warm chain"
                    )
                warm_prev = wmm

            sb = {}
            for name, cols in in_specs:
                sb[name] = consts.tile([128, cols], bf16, tag=name, name=name)
            nc.gpsimd.dma_start(out=sb["inp"], in_=ins["inp"])

            def tt_cols(c0, cw):
                """weight slice [128, cw] at global tt column c0"""
                return sb["inp"][:, c0 : c0 + cw]

            def av_cols(v0, vw):
                o = 3 * NU_SHARD + v0
                return sb["inp"][:, o : o + vw]

            # S[u, v, d] = sum_j Tt_d[j, u] * Av[v, j]
            copy_engines = [nc.vector.tensor_copy, nc.scalar.copy]
            n_copy = 0
            prev_mm = warm_prev
            prev_dma = None
            ob = obuf.tile([128, 6 * N_EVAL], bf16, tag="ob")
            for t, (u0, uw) in enumerate(U_TILES):
                for d in range(3):
                    p = 3 * t + d
                    w = tt_cols(d * NU_SHARD + u0, uw)
                    ldw = nc.tensor.ldweights(w)
                    if prev_mm is not None:
                        tile.add_dep_helper(
                            ldw.ins, prev_mm.ins, sync=False,
                            reason="weight group order",
                        )
                    for v0 in range(0, N_EVAL, V_TILE):
                        vw = min(V_TILE, N_EVAL - v0)
                        ps = ps2.tile([128, V_TILE], f32, tag="ps")
                        mm = nc.tensor.matmul(
                            ps[:uw, :vw], w, av_cols(v0, vw),
                            start=True, stop=True,
                        )
                        mm.ins.ldweights = False
                        tile.add_dep_helper(
                            mm.ins, ldw.ins, sync=False,
                            reason="matmul after its ldweights",
                        )
                        prev_mm = mm
                        # alternate DVE/ACT cast-copies; disjoint regions
                        # run concurrently (wide 2-bank copies measured a
                        # wash: less per-copy overhead but a coarser ramp)
                        copy_engines[n_copy % 2](
                            ob[:uw, p * N_EVAL + v0 : p * N_EVAL + v0 + vw],
                            ps[:uw, :vw],
                        )
                        n_copy += 1
                    # Two chained SWDGE DMAs, straight SBUF image -> DRAM:
                    # a 1 MB head (planes 0-1) early, then the 2 MB rest.
                    # The second waits for the first's COMPLETION so at
                    # most one DMA is ever outstanding (two coexisting
                    # DMAs latch the SDMA subsystem to ~83 GB/s until it
                    # drains); the head's ~2 us completion-receipt tail
                    # hides under the remaining copies.
                    p = 3 * t + d
                    if p == 1 or p == 5:
                        csl = (slice(0, 2 * N_EVAL) if p == 1
                               else slice(2 * N_EVAL, 6 * N_EVAL))
                        dma = nc.gpsimd.dma_start(
                            out=out_h[:, csl], in_=ob[:, csl]
                        )
                        if prev_dma is not None:
                            tile.add_dep_helper(
                                dma.ins, prev_dma.ins, sync=True,
                                reason="one output DMA in flight at a time",
                            )
                        prev_dma = dma


    nc.compile()
    _CACHE["nc"] = nc
    return nc


def kernel(control_points, params_u, params_v, degree):
    from concourse.bass_utils import run_bass_kernel_spmd

    p = int(np.asarray(degree))
    cp = np.asarray(control_points, np.float32)
    pu = np.asarray(params_u, np.float32)
    pv = np.asarray(params_v, np.float32)
    assert cp.shape == (N_CTRL, N_CTRL, 3), cp.shape
    assert pu.shape == (N_EVAL,) and pv.shape == (N_EVAL,), (pu.shape, pv.shape)

    Au = np.zeros((N_CORES * NU_SHARD, N_CTRL), np.float64)
    Au[:N_EVAL] = _dense_basis(pu, p, N_CTRL)
    Av = _dense_basis(pv, p, N_CTRL)

    # host stage 1 (0.3% of the FLOPs): T[j, d, u] = sum_i P[i,j,d] Au[u,i]
    T = (cp.astype(np.float64).transpose(1, 2, 0).reshape(3 * N_CTRL, N_CTRL)
         @ Au.T).reshape(N_CTRL, 3, N_CORES * NU_SHARD)

    av = _dup_halves_bf16(Av.T)  # [128, 2001]

    nc = _build_device()
    in_maps = []
    for c in range(N_CORES):
        tt = _stack_hilo(
            T[:, :, c * NU_SHARD : (c + 1) * NU_SHARD]
            .reshape(N_CTRL, -1).astype(np.float32)
        )
        in_maps.append({
            "inp": np.ascontiguousarray(np.concatenate([tt, av], axis=1)),
        })

    res = run_bass_kernel_spmd(
        nc,
        in_maps,
        core_ids=list(range(N_CORES)),
        trace=_CACHE.get("trace", False),
        **_CACHE.get("run_kwargs", {}),
    )
    _CACHE["last_result"] = res
    # DRAM row q block p = S[128*u_tile + q, :, d] for p = 3*u_tile + d
    slabs = []
    for r in res.results:
        arr = np.asarray(r["out"]).reshape(128, 6, N_EVAL)
        for t, (u0, uw) in enumerate(U_TILES):
            slabs.append(arr[:uw, 3 * t : 3 * t + 3])  # [uw, 3, N_EVAL]
    full = np.concatenate(slabs, axis=0)[:N_EVAL]  # [Nu, 3, Nv]
    return np.ascontiguousarray(full.transpose(0, 2, 1).astype(np.float32))


# revision 68
# speedup vs baseline: 1.0710x; 1.0710x over previous
"""B-spline (clamped) surface evaluation on 8 Trainium2 cores.

Math: out[u, v, :] = sum_{a,b} Bu[u,a] * Bv[v,b] * P[su[u]-p+a, sv[v]-p+b, :]

Host precomputes the tiny Cox-de-Boor basis, scatters it into dense matrices
Au [Nu, 64], Av [Nv, 64], and folds the small control-point contraction
T[u, j, d] = sum_i Au[u, i] P[i, j, d] (25M MACs, fp64 on host). The device
then does the dominant contraction (768M MACs):

  S[u, v, d] = sum_j T[u, j, d] * Av[v, j]       (TensorEngine matmuls)

The harness tolerance is 2e-2 rel; a single bf16 pass over Av (the rhs)
lands ~2.4e-3, so no second (Av_lo) pass is needed. The weights keep the
hi/lo split packed on the K axis (lhsT = [T_hi; T_lo], rhs = [Av; Av]) —
that costs nothing (matmul time scales with N only) and keeps the full
128x128 PE array active, which the HAM clock-gate needs to un-throttle the
PE from 1.2 to 2.4 GHz (observed: K=64 matmuls stay cold forever).

Schedule notes (from NTFF analysis on this platform):
- THE load-bearing discovery: a single outstanding SWDGE DMA streams at
  ~350-550 GB/s across all 16 SDMA engines, but the moment a second DMA
  coexists anywhere in the subsystem, throughput latches to ~83 GB/s
  (three engines' worth) until the rings drain. HWDGE stores never engage
  more than 3 engines at all. The output is therefore written by exactly
  TWO SWDGE DMAs (1 MB head once planes 0-1 are ready, then the 2 MB
  rest), the second chained on the first's COMPLETION semaphore, and the
  head's ~2 us completion-receipt hides under the remaining copies.
- Every DMA completion waits on all 16 per-engine semaphore increments;
  the last one lands ~2 us after the data. Inputs go out as a small head
  chunk (tt0: first weight group; av0: first v-tile) that unblocks the
  first matmuls earlier, with the rest arriving on the other HWDGE ring
  in parallel (merged/single-DMA input variants measured slower).
- Warm-up matmuls on a zeroed scratch tile bridge the preamble->input gap
  so the PE HAM activity window is busy before the real stream starts.
- PSUM->SBUF cast-copies alternate DVE/ACT (~690 ns per [128,512] fp32
  tile each; this paces the middle phase at ~8.4 us). GPSIMD cannot read
  PSUM on TRN2.
- Output DRAM layout is a straight SBUF image, partition-major: DRAM row
  q holds partition q's six plane-blocks [p0|..|p5] (plane p = 3*u_tile
  + d, row u = 128*u_tile + q), so every partition is one contiguous
  24 KB chunk.

Sharding: data-parallel over u. Each core computes a [251, 2001, 3] slab.
"""

import numpy as np

N_CTRL = 64
N_EVAL = 2001
N_CORES = 8
NU_SHARD = 251  # ceil(2001 / 8); 8 * 251 = 2008 (last 7 rows are zero padding)
U_TILES = [(0, 128), (128, NU_SHARD - 128)]
V_TILE = 512
N_WARMUP_MM = 6

_CACHE = {}


def _clamped_knots(p, n_ctrl, dtype=np.float64):
    n_internal = n_ctrl - p - 1
    internal = np.linspace(0.0, 1.0, n_internal + 2, dtype=dtype)[1:-1]
    return np.concatenate(
        [np.zeros(p + 1, dtype), internal, np.ones(p + 1, dtype)]
    )


def _dense_basis(params, p, n_ctrl):
    """Dense basis matrix A [len(params), n_ctrl], float64, with
    A[k, span-p+a] = B[k, a] (Cox-de-Boor, NURBS book A2.2)."""
    knots = _clamped_knots(p, n_ctrl)
    u = np.asarray(params, np.float64)
    spans = np.clip(np.searchsorted(knots, u, side="right") - 1, p, n_ctrl - 1)
    Ns = [np.ones_like(u)]
    left = {}
    right = {}
    for j in range(1, p + 1):
        left[j] = u - knots[spans + 1 - j]
        right[j] = knots[spans + j] - u
        saved = np.zeros_like(u)
        new = []
        for r in range(j):
            temp = Ns[r] / (right[r + 1] + left[j - r])
            new.append(saved + right[r + 1] * temp)
            saved = left[j - r] * temp
        new.append(saved)
        Ns = new
    B = np.stack(Ns, axis=-1)  # [N, p+1]
    A = np.zeros((len(u), n_ctrl), np.float64)
    rows = np.arange(len(u))[:, None]
    cols = spans[:, None] - p + np.arange(p + 1)[None, :]
    A[rows, cols] = B
    return A


def _split_bf16(a):
    """fp32 array -> (hi, lo) bf16 arrays with hi + lo ~= a (~2^-17 rel)."""
    import ml_dtypes

    a = np.ascontiguousarray(a, np.float32)
    hi = a.astype(ml_dtypes.bfloat16)
    lo = (a - hi.astype(np.float32)).astype(ml_dtypes.bfloat16)
    return hi, lo


def _stack_hilo(a):
    """[64, N] fp32 -> [128, N] bf16 with rows 0-63 = hi, 64-127 = lo."""
    hi, lo = _split_bf16(a)
    return np.ascontiguousarray(np.concatenate([hi, lo], axis=0))


def _dup_halves_bf16(a):
    """[64, N] fp32 -> [128, N] bf16 with the same data in both halves."""
    import ml_dtypes

    h = np.asarray(a, np.float32).astype(ml_dtypes.bfloat16)
    return np.ascontiguousarray(np.concatenate([h, h], axis=0))


def _build_device():
    if "nc" in _CACHE:
        return _CACHE["nc"]

    import concourse.mybir as mybir
    import concourse.tile as tile
    from concourse import bacc

    f32 = mybir.dt.float32
    bf16 = mybir.dt.bfloat16
    nc = bacc.Bacc(
        "TRN2", target_bir_lowering=False, debug=False, num_devices=N_CORES,
    )
    # ONE merged input DMA ([tt | av], 704 KB) on SWDGE ring 0: a solo
    # SWDGE DMA streams at full rate (~400 GB/s; HWDGE loads crawl at
    # ~150 GB/s and parallel split pieces trip the two-outstanding latch
    # with core-dependent stragglers). Data lands ~8.9 us, completion sem
    # ~11.2 us, UNIFORM across cores — and the graded time is the max.
    in_specs = [
        ("inp", 3 * NU_SHARD + N_EVAL),
    ]
    ins = {
        name: nc.dram_tensor(name, [128, cols], bf16, kind="ExternalInput").ap()
        for name, cols in in_specs
    }
    # ONE 3 MB output DMA at the end, partition-major: DRAM row q holds
    # partition q's six plane-blocks [p0|..|p5] (plane p = 3*u_tile + d,
    # row u = 128*u_tile + q), i.e. a straight SBUF image. A single
    # outstanding SWDGE DMA sustains ~350-430 GB/s, but the moment two
    # coexist in the ring the SDMA subsystem latches to ~83 GB/s (three
    # engines' worth) until it drains — one big DMA beats any pipelining.
    out_h = nc.dram_tensor(
        "out", [128, 6 * N_EVAL], bf16, kind="ExternalOutput"
    ).ap()

    with tile.TileContext(nc) as tc:
        with (
            tc.tile_pool(name="consts", bufs=1) as consts,
            tc.tile_pool(name="ps2", bufs=7, space="PSUM") as ps2,
            tc.tile_pool(name="warm", bufs=1, space="PSUM") as ps_warm,
            tc.tile_pool(name="obuf", bufs=2) as obuf,
        ):
            # PE warm-up on a zeroed scratch tile while the input DMA flies
            # (Tile refuses read-never-written tiles, so the memset stays)
            warm_in = consts.tile([128, V_TILE], bf16, tag="warm", name="warm")
            nc.vector.memset(warm_in, 0)
            pw = ps_warm.tile([128, V_TILE], f32, tag="psw")
            warm_prev = None
            for i in range(N_WARMUP_MM):
                wmm = nc.tensor.matmul(
                    pw[:128, :], warm_in[:, :128], warm_in, start=True, stop=True
                )
                if warm_prev is not None:
                    tile.add_dep_helper(
                        wmm.ins, warm_prev.ins, sync=False, reason="warm chain"
                    )
                warm_prev = wmm

            sb = {}
            for name, cols in in_specs:
                sb[name] = consts.tile([128, cols], bf16, tag=name, name=name)
            nc.gpsimd.dma_start(out=sb["inp"], in_=ins["inp"])

            def tt_cols(c0, cw):
                """weight slice [128, cw] at global tt column c0"""
                return sb["inp"][:, c0 : c0 + cw]

            def av_cols(v0, vw):
                o = 3 * NU_SHARD + v0
                return sb["inp"][:, o : o + vw]

            # S[u, v, d] = sum_j Tt_d[j, u] * Av[v, j]
            copy_engines = [nc.vector.tensor_copy, nc.scalar.copy]
            n_copy = 0
            prev_mm = warm_prev
            prev_dma = None
            ob = obuf.tile([128, 6 * N_EVAL], bf16, tag="ob")
            for t, (u0, uw) in enumerate(U_TILES):
                for d in range(3):
                    p = 3 * t + d
                    w = tt_cols(d * NU_SHARD + u0, uw)
                    ldw = nc.tensor.ldweights(w)
                    if prev_mm is not None:
                        tile.add_dep_helper(
                            ldw.ins, prev_mm.ins, sync=False,
                            reason="weight group order",
                        )
                    for v0 in range(0, N_EVAL, V_TILE):
                        vw = min(V_TILE, N_EVAL - v0)
                        ps = ps2.tile([128, V_TILE], f32, tag="ps")
                        mm = nc.tensor.matmul(
                            ps[:uw, :vw], w, av_cols(v0, vw),
                            start=True, stop=True,
                        )
                        mm.ins.ldweights = False
                        tile.add_dep_helper(
                            mm.ins, ldw.ins, sync=False,
                            reason="matmul after its ldweights",
                        )
                        prev_mm = mm
                        # alternate DVE/ACT cast-copies; disjoint regions
                        # run concurrently (wide 2-bank copies measured a
                        # wash: less per-copy overhead but a coarser ramp)
                        copy_engines[n_copy % 2](
                            ob[:uw, p * N_EVAL + v0 : p * N_EVAL + v0 + vw],
                            ps[:uw, :vw],
                        )
                        n_copy += 1
                    # Two chained SWDGE DMAs, straight SBUF image -> DRAM:
                    # a 1 MB head (planes 0-1) early, then the 2 MB rest.
                    # The second waits for the first's COMPLETION so at
                    # most one DMA is ever outstanding (two coexisting
                    # DMAs latch the SDMA subsystem to ~83 GB/s until it
                    # drains); the head's ~2 us completion-receipt tail
                    # hides under the remaining copies.
                    p = 3 * t + d
                    if p == 1 or p == 5:
                        csl = (slice(0, 2 * N_EVAL) if p == 1
                               else slice(2 * N_EVAL, 6 * N_EVAL))
                        dma = nc.gpsimd.dma_start(
                            out=out_h[:, csl], in_=ob[:, csl]
                        )
                        if prev_dma is not None:
                            tile.add_dep_helper(
                                dma.ins, prev_dma.ins, sync=True,
                                reason="one output DMA in flight at a time",
                            )
                        prev_dma = dma


    nc.compile()
    _CACHE["nc"] = nc
    return nc


def kernel(control_points, params_u, params_v, degree):
    from concourse.bass_utils import run_bass_kernel_spmd

    p = int(np.asarray(degree))
    cp = np.asarray(control_points, np.float32)
    pu = np.asarray(params_u, np.float32)
    pv = np.asarray(params_v, np.float32)
    assert cp.shape == (N_CTRL, N_CTRL, 3), cp.shape
    assert pu.shape == (N_EVAL,) and pv.shape == (N_EVAL,), (pu.shape, pv.shape)

    Au = np.zeros((N_CORES * NU_SHARD, N_CTRL), np.float64)
    Au[:N_EVAL] = _dense_basis(pu, p, N_CTRL)
    Av = _dense_basis(pv, p, N_CTRL)

    # host stage 1 (0.3% of the FLOPs): T[j, d, u] = sum_i P[i,j,d] Au[u,i]
    T = (cp.astype(np.float64).transpose(1, 2, 0).reshape(3 * N_CTRL, N_CTRL)
         @ Au.T).reshape(N_CTRL, 3, N_CORES * NU_SHARD)

    av = _dup_halves_bf16(Av.T)  # [128, 2001]

    nc = _build_device()
    in_maps = []
    for c in range(N_CORES):
        tt = _stack_hilo(
            T[:, :, c * NU_SHARD : (c + 1) * NU_SHARD]
            .reshape(N_CTRL, -1).astype(np.float32)
        )
        in_maps.append({
            "inp": np.ascontiguousarray(np.concatenate([tt, av], axis=1)),
        })

    res = run_bass_kernel_spmd(
        nc,
        in_maps,
        core_ids=list(range(N_CORES)),
        trace=_CACHE.get("trace", False),
        **_CACHE.get("run_kwargs", {}),
    )
    _CACHE["last_result"] = res
    # DRAM row q block p = S[128*u_tile + q, :, d] for p = 3*u_tile + d
    slabs = []
    for r in res.results:
        arr = np.asarray(r["out"]).reshape(128, 6, N_EVAL)
        for t, (u0, uw) in enumerate(U_TILES):
            slabs.append(arr[:uw, 3 * t : 3 * t + 3])  # [uw, 3, N_EVAL]
    full = np.concatenate(slabs, axis=0)[:N_EVAL]  # [Nu, 3, Nv]
    return np.ascontiguousarray(full.transpose(0, 2, 1).astype(np.float32))
